# revision 27
# baseline (speedup 1.0000x reference)
"""ConfidenceGate Trainium2 kernel (8 NeuronCores, SPMD) — v4.

Problem recap (shapes hardcoded from the spec):
  x:      (4, 512, 256, 7, 7) f32
  prev_x: (4, 512, 256, 7, 7) f32
  match:  (4, 512, 513) f32
  + tiny proj/LN/MLP params.
Reference returns c[0] -> (512, 1): only batch 0 contributes to the output.

Strategy (v4 — PE-fused streaming, col-tiled, fast epilogue):
  * Batch 0 only; data-parallel over M=512 rois: 8 cores x 64.
  * top1 = argmax(match[0,:,:512]) on host; prev rows pre-gathered per shard.
  * Pooling AND projection fused into accumulating matmuls on the Tensor
    engine: 98 blocks of [128 chan-half, 128 rois(64 x | 64 v)] in fp8
    (x*8, w*32 scaling; scales folded into the epilogue).  Weights
    ([128, 33] incl a negated-column-mean column for free centering) are
    stationary; even/odd blocks go to PE col-groups (0,0)/(0,64) so two
    streams run concurrently on separate sub-arrays.  proj_b rides in as
    a K=1 f32 rank-1 matmul into group A.
  * PE pre-warmed with junk matmuls (HAM un-throttle) during the DMA fill;
    gpsimd memsets run BEFORE its SWDGE dma_starts (the SWDGE drain
    otherwise serializes them behind DMA completion).
  * ln_b == 0 fast path: per-roi rstd cancels exactly inside the
    normalized cosine, so the LN variance/sqrt/scale chain is skipped
    (u = g*(v-mu) feeds the cosine directly).  General path kept for
    nonzero ln_b.
  * result is PE-transposed to a [1, 64] row so the output DMA is one
    contiguous 256B descriptor (64 partition-strided 4B writes cost ~6us
    in completion-semaphore latency).
  * match stats ([64,512] fp16) on DVE, overlapped with the stream; 4 of
    5 MLP features pre-accumulated.  ACT tables: Ln early, Sqrt primed
    mid-stream, Sigmoid load hidden under the MLP DVE ops.
"""

import sys

if "/opt/trn_rl_repo" not in sys.path:
    sys.path.insert(0, "/opt/trn_rl_repo")

import numpy as np
import ml_dtypes

B, M, N, C, G = 4, 512, 512, 256, 7
S = G * G                      # 49 spatial positions
PP, HH = 32, 32                # proj dim, MLP hidden
NCORES = 8
MS = M // NCORES               # 64 rois per core
CH = C // 2                    # 128 channels per half = contract dim
NBLK = 2 * S                   # 98 matmul blocks (h-major: b = h*49 + s)
BW = 2 * MS                    # 128 cols per block (64 x | 64 v)
NCHUNK = 7
BPC = NBLK // NCHUNK           # 14 blocks per chunk
SX, SW = 8.0, 32.0             # fp8 scales for data / weights
SEFF = S * SX * SW             # 12544 = total scale on psum values
EPS = 1e-9
LN_EPS = 1e-5
NEG = -60000.0                 # fp16-safe "-inf" for second-max masking
NPRE = 5                       # PE prewarm matmuls (N=512 each, ~0.63us cold)
COLT = True                    # 2x PE column tiling (groups at col 0 / 64)

F8 = ml_dtypes.float8_e4m3

# axf (f32) column layout, 64 partitions (one row per roi)
A_G = 0                        # ln_g / SEFF replicated twice (64, 64) [x|v]
A_B = 64                       # ln_b replicated twice (64, 64) for [x|v]
A_W1 = 128                     # w1 block f at [128+32f : 160+32f), f=0..4
A_B1 = 288                     # b1 replicated (64, 32)
A_W2 = 320                     # w2[0] replicated (64, 32)
A_B2 = 352                     # b2 (64, 1)
A_BIAS = 353                   # SEFF*(proj_b | -mean) (row 0 only, 33)
A_ONES = 386                   # 1.0 x 128 (row 0 only)
A_ID64 = 514                   # 64x64 identity (rows 0..63)
A_COLS = 578

_CACHE = {}


def _build(fastpath):
    import concourse.bacc as bacc
    import concourse.tile as tile
    import concourse.mybir as mybir

    dt = mybir.dt
    Alu = mybir.AluOpType
    Act = mybir.ActivationFunctionType
    Ax = mybir.AxisListType
    f32 = dt.float32
    f16 = dt.float16
    f8 = dt.float8e4

    nc = bacc.Bacc("TRN2", target_bir_lowering=False, debug=False)

    st_d = nc.dram_tensor("st", [CH, NBLK * BW], f8, kind="ExternalInput")
    mt_d = nc.dram_tensor("mt", [MS, N + 1], f16, kind="ExternalInput")
    wb_d = nc.dram_tensor("wb", [CH, 2 * (PP + 1)], f8, kind="ExternalInput")
    id_d = nc.dram_tensor("idn", [PP + 1, PP + 1], f32, kind="ExternalInput")
    axf_d = nc.dram_tensor("axf", [MS, A_COLS], f32, kind="ExternalInput")
    out_d = nc.dram_tensor("out", [1, MS], f32, kind="ExternalOutput")
    dbg_d = nc.dram_tensor("dbg", [1, 8], f32, kind="ExternalOutput")

    NG = 2 if COLT else 1      # psum col groups

    with tile.TileContext(nc) as tc, nc.allow_low_precision(
        reason="fp8 pool+proj stream + fp16 match stats; logit margin ~0.79 "
        "vs ~1e-3 observed logit err (validated on host)"
    ):
        with (
            tc.tile_pool(name="persist", bufs=1) as per,
            tc.tile_pool(name="chunks", bufs=1) as big,
            tc.tile_pool(name="scratch", bufs=1) as scr,
            tc.tile_pool(name="psum", bufs=1, space="PSUM") as psp,
        ):
            # ---- small loads on the scalar (ACT) HWDGE ring; wb first
            # (gates matmuls), then axf (bias/identities), then mt ----
            wb = per.tile([CH, 2 * (PP + 1)], f8)
            nc.scalar.dma_start(out=wb[:], in_=wb_d[:])
            axf = per.tile([MS, A_COLS], f32)
            nc.scalar.dma_start(out=axf[:], in_=axf_d[:])
            # mt rides FIRST on the sync queue: on the scalar queue it
            # finishes last (~16us) behind the round-robin with the stream
            mt = per.tile([MS, N + 1], f16)
            nc.sync.dma_start(out=mt[:], in_=mt_d[:])
            # 33x33 identity replicated at partitions 0-32 and 64-96 for
            # the group-B transposes (rhs must live on the B row range)
            idn2 = per.tile([CH - 31, PP + 1], f32)
            nc.scalar.dma_start(out=idn2[0:PP + 1, :], in_=id_d[:])
            if COLT:
                nc.scalar.dma_start(out=idn2[MS:MS + PP + 1, :], in_=id_d[:])

            # ---- gpsimd: ALL memsets FIRST (before its SWDGE dma_starts,
            # which drain-block anything queued after them) ----
            e9 = per.tile([MS, 1], f32)
            nc.gpsimd.memset(e9[:], EPS)
            dmy = scr.tile([1, 1], f32, tag="dmy")
            nc.gpsimd.memset(dmy[:], 1.0)
            pr_in = scr.tile([1, 8], f32, tag="prin")
            nc.gpsimd.memset(pr_in[:], 30.0)
            if not fastpath:
                eln = per.tile([MS, 1], f32)
                nc.gpsimd.memset(eln[:], LN_EPS)

            # ---- big stream chunks: 4 on sync HWDGE, 3 on gpsimd SWDGE ----
            chunks = []
            for i in range(NCHUNK):
                t = big.tile([CH, BPC * BW], f8, tag=f"ch{i}", name=f"ch{i}")
                c0 = i * BPC * BW
                eng = nc.sync if i < 4 else nc.gpsimd
                eng.dma_start(out=t[:], in_=st_d[:, c0:c0 + BPC * BW])
                chunks.append(t)

            # ---- PE prewarm (HAM un-throttle during DMA fill). junk is
            # memset on the DVE (idle until mt arrives) so prewarm starts
            # right after the preamble ----
            junk = per.tile([CH, 512], f16)
            nc.vector.memset(junk[:], 0.25)
            jps = psp.tile([PP + 1, 512], f32, tag="jps", name="jps")
            for _ in range(NPRE):
                nc.tensor.matmul(jps[:], junk[:, 0:PP + 1], junk[:],
                                 start=True, stop=True, skip_group_check=True)

            # ---- dbg sentinel ----
            prb = per.tile([1, 8], f32)
            nc.vector.tensor_scalar(prb[:], pr_in[:], 1.0, None, op0=Alu.mult)
            nc.scalar.dma_start(out=dbg_d[:], in_=prb[:])

            # ---- match stats on DVE (overlap the stream) ----
            real = mt[:, 0:N]
            pd = mt[:, N:N + 1]
            rmass = per.tile([MS, 1], f32)
            jr = scr.tile([MS, N], f16, tag="jr")
            nc.vector.tensor_scalar(jr[:], real, 1.0, 0.0, op0=Alu.mult,
                                    op1=Alu.add, accum_out=rmass[:])
            f_pmax = per.tile([MS, 1], f32)
            nc.vector.reduce_max(f_pmax[:], real, axis=Ax.X)
            eqm = scr.tile([MS, N], f16, tag="eqm")
            nc.vector.tensor_scalar(eqm[:], real, f_pmax[:], None,
                                    op0=Alu.is_equal)
            msk = scr.tile([MS, N], f16, tag="msk")
            nc.vector.scalar_tensor_tensor(msk[:], eqm[:], NEG, real,
                                           op0=Alu.mult, op1=Alu.add)
            m2 = per.tile([MS, 1], f32)
            nc.vector.reduce_max(m2[:], msk[:], axis=Ax.X)
            f_gap = per.tile([MS, 1], f32)
            nc.vector.tensor_tensor(f_gap[:], f_pmax[:], m2[:],
                                    op=Alu.subtract)
            # entropy feature is -ent = sum(real * ln(real+eps)) directly
            lnr = scr.tile([MS, N], f16, tag="lnr")
            nc.scalar.activation(lnr[:], real, Act.Ln, bias=e9[:])
            f_ent = per.tile([MS, 1], f32)
            je = scr.tile([MS, N], f16, tag="je")
            nc.vector.scalar_tensor_tensor(je[:], real, 1.0, lnr[:],
                                           op0=Alu.bypass, op1=Alu.mult,
                                           accum_out=f_ent[:])
            # prime the Sqrt + Sigmoid tables while the stream is flowing
            # (the ACT block has 4 table slots; if Ln/Sqrt/Sigmoid coexist,
            # no load lands on the critical tail)
            pre = scr.tile([1, 2], f32, tag="pre")
            nc.scalar.activation(pre[:, 0:1], dmy[:], Act.Sqrt)
            nc.scalar.activation(pre[:, 1:2], dmy[:], Act.Sigmoid)
            f_pd = per.tile([MS, 1], f32)
            nc.vector.tensor_scalar(f_pd[:], pd, -1.0, 1.0, op0=Alu.mult,
                                    op1=Alu.add)
            hr9 = per.tile([MS, 1], f32)
            nc.vector.tensor_scalar(hr9[:], rmass[:], EPS, None, op0=Alu.is_gt)
            hr6 = per.tile([MS, 1], f32)
            nc.vector.tensor_scalar(hr6[:], rmass[:], 1e-6, None,
                                    op0=Alu.is_gt)

            # ---- MLP hidden pre-accumulation over the 4 early features ----
            hA = scr.tile([MS, HH], f32, tag="hA")
            nc.vector.scalar_tensor_tensor(
                hA[:], axf[:, A_W1:A_W1 + HH], f_pd[:],
                axf[:, A_B1:A_B1 + HH], op0=Alu.mult, op1=Alu.add)
            hB = scr.tile([MS, HH], f32, tag="hB")
            nc.vector.scalar_tensor_tensor(
                hB[:], axf[:, A_W1 + HH:A_W1 + 2 * HH], f_pmax[:], hA[:],
                op0=Alu.mult, op1=Alu.add)
            hC = scr.tile([MS, HH], f32, tag="hC")
            nc.vector.scalar_tensor_tensor(
                hC[:], axf[:, A_W1 + 2 * HH:A_W1 + 3 * HH], f_gap[:], hB[:],
                op0=Alu.mult, op1=Alu.add)
            hD = per.tile([MS, HH], f32, tag="hD")
            nc.vector.scalar_tensor_tensor(
                hD[:], axf[:, A_W1 + 3 * HH:A_W1 + 4 * HH], f_ent[:], hC[:],
                op0=Alu.mult, op1=Alu.add)

            # ---- the fused pool+proj matmul chain ----
            # ps1 partitions [0:33] = group A, [64:97] = group B.
            # The f32 proj_b rank-1 update runs LAST (it waits on the big
            # axf DMA; putting it first stalled the whole chain ~4.4us and
            # let the HAM re-throttle the PE).
            ps1 = psp.tile([CH, BW], f32, tag="ps1", name="ps1")
            started = [False, False]
            for b in range(NBLK):
                h = b // S
                ci, co = b // BPC, (b % BPC) * BW
                grp = (b % 2) if COLT else 0
                po = 0 if grp == 0 else MS
                st_flag = not started[grp]
                started[grp] = True
                # group B (odd) closes at b=97; group A closes at the bias MM
                stop_flag = COLT and (b == NBLK - 1)
                nc.tensor.matmul(
                    ps1[po:po + PP + 1, :],
                    wb[:, h * (PP + 1):(h + 1) * (PP + 1)],
                    chunks[ci][:, co:co + BW],
                    start=st_flag, stop=stop_flag,
                    skip_group_check=True,
                    tile_position=(0, po) if COLT else None)
            # bias into group A, closing its accumulation
            nc.tensor.matmul(ps1[0:PP + 1, :],
                             axf[0:1, A_BIAS:A_BIAS + PP + 1],
                             axf[0:1, A_ONES:A_ONES + BW],
                             start=False, stop=True, skip_group_check=True,
                             tile_position=(0, 0) if COLT else None)

            # ---- psum -> SBUF -> PE-transpose -> ps2[64, NG*2*33] ----
            sb1 = per.tile([CH - 31, BW], f32)
            nc.vector.tensor_copy(sb1[0:PP + 1, :], ps1[0:PP + 1, :])
            if COLT:
                nc.vector.tensor_copy(sb1[MS:MS + PP + 1, :],
                                      ps1[MS:MS + PP + 1, :])
            ps2 = psp.tile([MS, 2 * (PP + 1)], f32, tag="ps2", name="ps2")
            W33 = PP + 1
            # group-B transposes ACCUMULATE onto group A's region, merging
            # the col-tiled halves for free on the PE
            nc.tensor.matmul(ps2[:, 0:W33], sb1[0:W33, 0:MS],
                             idn2[0:W33, :], is_transpose=True,
                             start=True, stop=not COLT,
                             skip_group_check=True, tile_position=(0, 0))
            nc.tensor.matmul(ps2[:, W33:2 * W33], sb1[0:W33, MS:BW],
                             idn2[0:W33, :], is_transpose=True,
                             start=True, stop=not COLT,
                             skip_group_check=True, tile_position=(0, 0))
            if COLT:
                nc.tensor.matmul(ps2[:, 0:W33], sb1[MS:MS + W33, 0:MS],
                                 idn2[MS:MS + W33, :], is_transpose=True,
                                 start=False, stop=True,
                                 skip_group_check=True,
                                 tile_position=(MS, 0))
                nc.tensor.matmul(ps2[:, W33:2 * W33],
                                 sb1[MS:MS + W33, MS:BW],
                                 idn2[MS:MS + W33, :], is_transpose=True,
                                 start=False, stop=True,
                                 skip_group_check=True,
                                 tile_position=(MS, 0))

            # ---- center:  src cols = [v | -mu] per roi, per w-group ----
            src = ps2
            ctr = per.tile([MS, 2 * PP], f32)
            for w in (0, 1):
                po = w * W33
                nc.vector.tensor_scalar(ctr[:, w * PP:(w + 1) * PP],
                                        src[:, po:po + PP],
                                        src[:, po + PP:po + PP + 1],
                                        None, op0=Alu.add)

            if fastpath:
                # ln_b == 0: rstd cancels in the normalized cosine, so
                # cos = <g*ctr_x, g*ctr_v> / (|g*ctr_x| |g*ctr_v|)
                u = per.tile([MS, 2 * PP], f32)
                nc.vector.tensor_tensor(u[:], ctr[:],
                                        axf[:, A_G:A_G + 2 * PP],
                                        op=Alu.mult)
                ss = per.tile([MS, 2], f32)
                jn = scr.tile([MS, 2 * PP], f32, tag="jn")
                for w in (0, 1):
                    us = u[:, w * PP:(w + 1) * PP]
                    nc.vector.scalar_tensor_tensor(
                        jn[:, w * PP:(w + 1) * PP], us, 1.0, us,
                        op0=Alu.bypass, op1=Alu.mult,
                        accum_out=ss[:, w:w + 1])
                dot = per.tile([MS, 1], f32)
                jd = scr.tile([MS, PP], f32, tag="jd")
                nc.vector.scalar_tensor_tensor(jd[:], u[:, 0:PP], 1.0,
                                               u[:, PP:2 * PP],
                                               op0=Alu.bypass, op1=Alu.mult,
                                               accum_out=dot[:])
            else:
                vs = per.tile([MS, 2], f32)
                jv = scr.tile([MS, 2 * PP], f32, tag="jv")
                for w in (0, 1):
                    cs = ctr[:, w * PP:(w + 1) * PP]
                    nc.vector.scalar_tensor_tensor(
                        jv[:, w * PP:(w + 1) * PP], cs, 1.0, cs,
                        op0=Alu.bypass, op1=Alu.mult,
                        accum_out=vs[:, w:w + 1])
                sd = scr.tile([MS, 2], f32, tag="sd")
                nc.scalar.activation(sd[:], vs[:], Act.Sqrt,
                                     scale=1.0 / (PP * SEFF * SEFF),
                                     bias=eln[:])
                rstd = per.tile([MS, 2], f32)
                nc.vector.reciprocal(rstd[:], sd[:])
                gr = scr.tile([MS, 2 * PP], f32, tag="gr")
                for w in (0, 1):
                    nc.vector.tensor_scalar(gr[:, w * PP:(w + 1) * PP],
                                            axf[:, A_G + w * PP:
                                                A_G + (w + 1) * PP],
                                            rstd[:, w:w + 1], None,
                                            op0=Alu.mult)
                yt = scr.tile([MS, 2 * PP], f32, tag="yt")
                nc.vector.tensor_tensor(yt[:], ctr[:], gr[:], op=Alu.mult)
                u = per.tile([MS, 2 * PP], f32)
                nc.vector.tensor_tensor(u[:], yt[:],
                                        axf[:, A_B:A_B + 2 * PP], op=Alu.add)
                ss = per.tile([MS, 2], f32)
                jn = scr.tile([MS, 2 * PP], f32, tag="jn")
                for w in (0, 1):
                    ys = u[:, w * PP:(w + 1) * PP]
                    nc.vector.scalar_tensor_tensor(
                        jn[:, w * PP:(w + 1) * PP], ys, 1.0, ys,
                        op0=Alu.bypass, op1=Alu.mult,
                        accum_out=ss[:, w:w + 1])
                dot = per.tile([MS, 1], f32)
                jd = scr.tile([MS, PP], f32, tag="jd")
                nc.vector.scalar_tensor_tensor(jd[:], u[:, 0:PP], 1.0,
                                               u[:, PP:2 * PP],
                                               op0=Alu.bypass, op1=Alu.mult,
                                               accum_out=dot[:])

            s12 = scr.tile([MS, 1], f32, tag="s12")
            nc.vector.tensor_tensor(s12[:], ss[:, 0:1], ss[:, 1:2],
                                    op=Alu.mult)
            sq = scr.tile([MS, 1], f32, tag="sq")
            nc.scalar.activation(sq[:], s12[:], Act.Sqrt)
            rq = per.tile([MS, 1], f32)
            nc.vector.reciprocal(rq[:], sq[:])
            f_cos = per.tile([MS, 1], f32)
            nc.vector.scalar_tensor_tensor(f_cos[:], dot[:], rq[:], hr9[:],
                                           op0=Alu.mult, op1=Alu.mult)

            # ---- finish MLP ----
            hE = per.tile([MS, HH], f32)
            nc.vector.scalar_tensor_tensor(
                hE[:], axf[:, A_W1 + 4 * HH:A_W1 + 5 * HH], f_cos[:],
                hD[:], op0=Alu.mult, op1=Alu.add)
            hR = per.tile([MS, HH], f32)
            nc.vector.tensor_scalar(hR[:], hE[:], 0.0, None, op0=Alu.max)
            logit = per.tile([MS, 1], f32)
            jl = scr.tile([MS, HH], f32, tag="jl")
            nc.vector.scalar_tensor_tensor(jl[:], hR[:], 1.0,
                                           axf[:, A_W2:A_W2 + HH],
                                           op0=Alu.bypass, op1=Alu.mult,
                                           accum_out=logit[:])
            sg = per.tile([MS, 1], f32)
            nc.scalar.activation(sg[:], logit[:], Act.Sigmoid,
                                 bias=axf[:, A_B2:A_B2 + 1])
            gt = per.tile([MS, 1], f32)
            nc.vector.scalar_tensor_tensor(gt[:], sg[:], 0.999, hr6[:],
                                           op0=Alu.min, op1=Alu.mult)
            res = per.tile([MS, 1], f32)
            nc.vector.tensor_scalar(res[:], gt[:], 0.001, None, op0=Alu.max)

            # ---- transpose result to a [1, 64] row -> single contiguous
            # 256B output DMA (partition-strided 4B writes stall the
            # completion semaphore ~6us) ----
            pout = psp.tile([1, MS], f32, tag="pout", name="pout")
            nc.tensor.matmul(pout[:], res[:],
                             axf[:, A_ID64:A_ID64 + MS],
                             start=True, stop=True, skip_group_check=True)
            rrow = per.tile([1, MS], f32)
            nc.vector.tensor_copy(rrow[:], pout[:])
            nc.sync.dma_start(out=out_d[:], in_=rrow[:])

    nc.finalize()
    return nc


def _get_nc(fastpath):
    key = ("nc", fastpath)
    if key not in _CACHE:
        _CACHE[key] = _build(fastpath)
    return _CACHE[key]


def make_in_maps(x, prev_x, match, proj_w, proj_b, ln_g, ln_b, w1, b1, w2, b2):
    f32 = np.float32
    f16 = np.float16
    x0 = np.asarray(x[0], dtype=f32).reshape(M, C, S)
    p0 = np.asarray(prev_x[0], dtype=f32).reshape(N, C, S)
    mt0 = np.ascontiguousarray(np.asarray(match[0], dtype=f32))
    real0 = mt0[:, :N]
    rm = real0.sum(axis=1)
    top1 = np.where(rm > EPS, np.argmax(real0, axis=1), 0)

    proj_w = np.asarray(proj_w, dtype=f32)   # (32, 256)
    proj_b = np.asarray(proj_b, dtype=f32)

    # stream: [core, 128 chan-half, 98 blocks (h-major) x 128 (64 x | 64 v)]
    def shard_blocks(rows):                  # (512, 256, 49) -> (8,2,49,128,64)
        return (rows.reshape(NCORES, MS, 2, CH, S)
                    .transpose(0, 2, 4, 3, 1))
    xt = shard_blocks(x0 * SX)
    vt = shard_blocks(p0[top1] * SX)
    comb = np.concatenate([xt, vt], axis=4)              # (8,2,49,128,128)
    stream = np.ascontiguousarray(
        comb.transpose(0, 3, 1, 2, 4).reshape(NCORES, CH, NBLK * BW)
    ).astype(F8)

    # weights: per half h, [128, 33]: cols 0:32 = 32*w[:, h*128+c].T,
    # col 32 = -32 * mean_p w  (negated column-mean row for centering)
    wb = np.zeros((CH, 2 * (PP + 1)), dtype=f32)
    for h in (0, 1):
        blk = proj_w[:, h * CH:(h + 1) * CH].T * SW      # (128, 32)
        wb[:, h * (PP + 1):h * (PP + 1) + PP] = blk
        wb[:, h * (PP + 1) + PP] = -blk.mean(axis=1)
    wb = wb.astype(F8)

    idn = np.eye(PP + 1, dtype=f32)

    axf = np.zeros((MS, A_COLS), dtype=f32)
    ln_g = np.asarray(ln_g, dtype=f32)
    ln_b = np.asarray(ln_b, dtype=f32)
    fastpath = bool(np.all(ln_b == 0.0))
    # fastpath cosine is scale-invariant -> raw ln_g; general path folds
    # the stream scale into g (y = ctrS * rstd_true * g/SEFF)
    gfill = ln_g if fastpath else ln_g / SEFF
    axf[:, A_G:A_G + PP] = gfill
    axf[:, A_G + PP:A_G + 2 * PP] = gfill
    axf[:, A_B:A_B + PP] = ln_b
    axf[:, A_B + PP:A_B + 2 * PP] = ln_b
    w1 = np.asarray(w1, dtype=f32)           # (32, 5)
    for f in range(5):
        axf[:, A_W1 + f * HH:A_W1 + (f + 1) * HH] = w1[:, f]
    axf[:, A_B1:A_B1 + HH] = np.asarray(b1, dtype=f32)
    axf[:, A_W2:A_W2 + HH] = np.asarray(w2, dtype=f32)[0]
    axf[:, A_B2] = np.asarray(b2, dtype=f32)[0]
    axf[0, A_BIAS:A_BIAS + PP] = SEFF * proj_b
    axf[0, A_BIAS + PP] = -SEFF * proj_b.mean()
    axf[0, A_ONES:A_ONES + BW] = 1.0
    axf[:, A_ID64:A_ID64 + MS] = np.eye(MS, dtype=f32)

    in_maps = []
    for i in range(NCORES):
        lo, hi = i * MS, (i + 1) * MS
        in_maps.append({
            "st": stream[i],
            "mt": np.ascontiguousarray(mt0[lo:hi]).astype(f16),
            "wb": wb, "idn": idn, "axf": axf,
        })
    return in_maps, fastpath


def run(in_maps, fastpath=True, trace=False):
    from concourse.bass_utils import run_bass_kernel_spmd
    res = run_bass_kernel_spmd(_get_nc(fastpath), in_maps,
                               list(range(NCORES)), trace=trace)
    out = np.concatenate(
        [res.results[i]["out"].reshape(MS, 1) for i in range(NCORES)], axis=0)
    if trace:
        print("dbg sentinel (expect 30s):", res.results[0]["dbg"])
    return out.astype(np.float32), res


def kernel(x, prev_x, match, proj_w, proj_b, ln_g, ln_b, w1, b1, w2, b2):
    in_maps, fastpath = make_in_maps(x, prev_x, match, proj_w, proj_b,
                                     ln_g, ln_b, w1, b1, w2, b2)
    out, _ = run(in_maps, fastpath=fastpath, trace=False)
    return out


# revision 32
# speedup vs baseline: 1.1264x; 1.1264x over previous
"""ConfidenceGate Trainium2 kernel (8 NeuronCores, SPMD) — v4.

Problem recap (shapes hardcoded from the spec):
  x:      (4, 512, 256, 7, 7) f32
  prev_x: (4, 512, 256, 7, 7) f32
  match:  (4, 512, 513) f32
  + tiny proj/LN/MLP params.
Reference returns c[0] -> (512, 1): only batch 0 contributes to the output.

Strategy (v4 — PE-fused streaming, col-tiled, fast epilogue):
  * Batch 0 only; data-parallel over M=512 rois: 8 cores x 64.
  * top1 = argmax(match[0,:,:512]) on host; prev rows pre-gathered per shard.
  * Pooling AND projection fused into accumulating matmuls on the Tensor
    engine: 98 blocks of [128 chan-half, 128 rois(64 x | 64 v)] in fp8
    (x*8, w*32 scaling; scales folded into the epilogue).  Weights
    ([128, 33] incl a negated-column-mean column for free centering) are
    stationary; even/odd blocks go to PE col-groups (0,0)/(0,64) so two
    streams run concurrently on separate sub-arrays.  proj_b rides in as
    a K=1 f32 rank-1 matmul into group A.
  * PE pre-warmed with junk matmuls (HAM un-throttle) during the DMA fill;
    gpsimd memsets run BEFORE its SWDGE dma_starts (the SWDGE drain
    otherwise serializes them behind DMA completion).
  * ln_b == 0 fast path: per-roi rstd cancels exactly inside the
    normalized cosine, so the LN variance/sqrt/scale chain is skipped
    (u = g*(v-mu) feeds the cosine directly).  General path kept for
    nonzero ln_b.
  * result is PE-transposed to a [1, 64] row so the output DMA is one
    contiguous 256B descriptor (64 partition-strided 4B writes cost ~6us
    in completion-semaphore latency).
  * match stats ([64,512] fp16) on DVE, overlapped with the stream; 4 of
    5 MLP features pre-accumulated.  ACT tables: Ln early, Sqrt primed
    mid-stream, Sigmoid load hidden under the MLP DVE ops.
"""

import sys

if "/opt/trn_rl_repo" not in sys.path:
    sys.path.insert(0, "/opt/trn_rl_repo")

import numpy as np
import ml_dtypes

B, M, N, C, G = 4, 512, 512, 256, 7
S = G * G                      # 49 spatial positions
PP, HH = 32, 32                # proj dim, MLP hidden
NCORES = 8
MS = M // NCORES               # 64 rois per core
CH = C // 2                    # 128 channels per half = contract dim
NBLK = 2 * S                   # 98 matmul blocks (h-major: b = h*49 + s)
BW = 2 * MS                    # 128 cols per block (64 x | 64 v)
NCHUNK = 7
BPC = NBLK // NCHUNK           # 14 blocks per chunk
SX, SW = 8.0, 32.0             # fp8 scales for data / weights
SEFF = S * SX * SW             # 12544 = total scale on psum values
EPS = 1e-9
LN_EPS = 1e-5
NEG = -60000.0                 # fp16-safe "-inf" for second-max masking
NPRE = 4                       # PE prewarm matmuls (N=512 each, ~0.75us cold)
COLT = True                    # 2x PE column tiling (groups at col 0 / 64)

F8 = ml_dtypes.float8_e4m3

# axf (f32) column layout, 64 partitions (one row per roi)
A_G = 0                        # ln_g / SEFF replicated twice (64, 64) [x|v]
A_B = 64                       # ln_b replicated twice (64, 64) for [x|v]
A_W1 = 128                     # w1 block f at [128+32f : 160+32f), f=0..4
A_B1 = 288                     # b1 replicated (64, 32)
A_W2 = 320                     # w2[0] replicated (64, 32)
A_B2 = 352                     # b2 (64, 1)
A_BIAS = 353                   # SEFF*(proj_b | -mean) (row 0 only, 33)
A_ONES = 386                   # 1.0 x 128 (row 0 only)
A_ID64 = 514                   # 64x64 identity (rows 0..63)
A_COLS = 578

_CACHE = {}


def _build(fastpath):
    import concourse.bacc as bacc
    import concourse.tile as tile
    import concourse.mybir as mybir

    dt = mybir.dt
    Alu = mybir.AluOpType
    Act = mybir.ActivationFunctionType
    Ax = mybir.AxisListType
    f32 = dt.float32
    f16 = dt.float16
    f8 = dt.float8e4

    nc = bacc.Bacc("TRN2", target_bir_lowering=False, debug=False)

    st_d = nc.dram_tensor("st", [CH, NBLK * BW], f8, kind="ExternalInput")
    mt_d = nc.dram_tensor("mt", [MS, N + 1], f16, kind="ExternalInput")
    wb_d = nc.dram_tensor("wb", [CH, 2 * (PP + 1)], f8, kind="ExternalInput")
    id_d = nc.dram_tensor("idn", [PP + 1, PP + 1], f32, kind="ExternalInput")
    axf_d = nc.dram_tensor("axf", [MS, A_COLS], f32, kind="ExternalInput")
    out_d = nc.dram_tensor("out", [1, MS], f32, kind="ExternalOutput")
    dbg_d = nc.dram_tensor("dbg", [1, 8], f32, kind="ExternalOutput")

    NG = 2 if COLT else 1      # psum col groups

    with tile.TileContext(nc) as tc, nc.allow_low_precision(
        reason="fp8 pool+proj stream + fp16 match stats; logit margin ~0.79 "
        "vs ~1e-3 observed logit err (validated on host)"
    ):
        with (
            tc.tile_pool(name="persist", bufs=1) as per,
            tc.tile_pool(name="chunks", bufs=1) as big,
            tc.tile_pool(name="scratch", bufs=1) as scr,
            tc.tile_pool(name="psum", bufs=1, space="PSUM") as psp,
        ):
            # ---- small loads on the scalar (ACT) HWDGE ring; wb first
            # (gates matmuls), then axf (bias/identities), then mt ----
            wb = per.tile([CH, 2 * (PP + 1)], f8)
            nc.scalar.dma_start(out=wb[:], in_=wb_d[:])
            axf = per.tile([MS, A_COLS], f32)
            nc.scalar.dma_start(out=axf[:], in_=axf_d[:])
            # mt rides FIRST on the sync queue: on the scalar queue it
            # finishes last (~16us) behind the round-robin with the stream
            mt = per.tile([MS, N + 1], f16)
            nc.sync.dma_start(out=mt[:], in_=mt_d[:])
            # 33x33 identity replicated at partitions 0-32 and 64-96 for
            # the group-B transposes (rhs must live on the B row range)
            idn2 = per.tile([CH - 31, PP + 1], f32)
            nc.scalar.dma_start(out=idn2[0:PP + 1, :], in_=id_d[:])
            if COLT:
                nc.scalar.dma_start(out=idn2[MS:MS + PP + 1, :], in_=id_d[:])

            # ---- gpsimd: ALL memsets FIRST (before its SWDGE dma_starts,
            # which drain-block anything queued after them) ----
            e9 = per.tile([MS, 1], f32)
            nc.gpsimd.memset(e9[:], EPS)
            dmy = scr.tile([1, 1], f32, tag="dmy")
            nc.gpsimd.memset(dmy[:], 1.0)
            pr_in = scr.tile([1, 8], f32, tag="prin")
            nc.gpsimd.memset(pr_in[:], 30.0)
            if not fastpath:
                eln = per.tile([MS, 1], f32)
                nc.gpsimd.memset(eln[:], LN_EPS)

            # ---- big stream chunks: 4 on sync HWDGE, 3 on gpsimd SWDGE ----
            chunks = []
            for i in range(NCHUNK):
                t = big.tile([CH, BPC * BW], f8, tag=f"ch{i}", name=f"ch{i}")
                c0 = i * BPC * BW
                eng = nc.sync if i < 4 else nc.gpsimd
                eng.dma_start(out=t[:], in_=st_d[:, c0:c0 + BPC * BW])
                chunks.append(t)

            # ---- PE prewarm (HAM un-throttle during DMA fill). junk is
            # memset on the DVE (idle until mt arrives) so prewarm starts
            # right after the preamble ----
            junk = per.tile([CH, 512], f16)
            nc.vector.memset(junk[:], 0.25)
            jps = psp.tile([PP + 1, 512], f32, tag="jps", name="jps")
            for _ in range(NPRE):
                nc.tensor.matmul(jps[:], junk[:, 0:PP + 1], junk[:],
                                 start=True, stop=True, skip_group_check=True)

            # ---- dbg sentinel ----
            prb = per.tile([1, 8], f32)
            nc.vector.tensor_scalar(prb[:], pr_in[:], 1.0, None, op0=Alu.mult)
            nc.scalar.dma_start(out=dbg_d[:], in_=prb[:])

            # ---- match stats on DVE (overlap the stream) ----
            real = mt[:, 0:N]
            pd = mt[:, N:N + 1]
            rmass = per.tile([MS, 1], f32)
            jr = scr.tile([MS, N], f16, tag="jr")
            nc.vector.tensor_scalar(jr[:], real, 1.0, 0.0, op0=Alu.mult,
                                    op1=Alu.add, accum_out=rmass[:])
            f_pmax = per.tile([MS, 1], f32)
            nc.vector.reduce_max(f_pmax[:], real, axis=Ax.X)
            eqm = scr.tile([MS, N], f16, tag="eqm")
            nc.vector.tensor_scalar(eqm[:], real, f_pmax[:], None,
                                    op0=Alu.is_equal)
            msk = scr.tile([MS, N], f16, tag="msk")
            nc.vector.scalar_tensor_tensor(msk[:], eqm[:], NEG, real,
                                           op0=Alu.mult, op1=Alu.add)
            m2 = per.tile([MS, 1], f32)
            nc.vector.reduce_max(m2[:], msk[:], axis=Ax.X)
            f_gap = per.tile([MS, 1], f32)
            nc.vector.tensor_tensor(f_gap[:], f_pmax[:], m2[:],
                                    op=Alu.subtract)
            # entropy feature is -ent = sum(real * ln(real+eps)) directly
            lnr = scr.tile([MS, N], f16, tag="lnr")
            nc.scalar.activation(lnr[:], real, Act.Ln, bias=e9[:])
            f_ent = per.tile([MS, 1], f32)
            je = scr.tile([MS, N], f16, tag="je")
            nc.vector.scalar_tensor_tensor(je[:], real, 1.0, lnr[:],
                                           op0=Alu.bypass, op1=Alu.mult,
                                           accum_out=f_ent[:])
            # prime the Sqrt table while the stream is flowing (priming
            # Sigmoid too backfires: 4 live sets -> walrus emits 6 loads)
            pre = scr.tile([1, 1], f32, tag="pre")
            nc.scalar.activation(pre[:], dmy[:], Act.Sqrt)
            f_pd = per.tile([MS, 1], f32)
            nc.vector.tensor_scalar(f_pd[:], pd, -1.0, 1.0, op0=Alu.mult,
                                    op1=Alu.add)
            hr9 = per.tile([MS, 1], f32)
            nc.vector.tensor_scalar(hr9[:], rmass[:], EPS, None, op0=Alu.is_gt)
            hr6 = per.tile([MS, 1], f32)
            nc.vector.tensor_scalar(hr6[:], rmass[:], 1e-6, None,
                                    op0=Alu.is_gt)

            # ---- MLP hidden pre-accumulation over the 4 early features ----
            hA = scr.tile([MS, HH], f32, tag="hA")
            nc.vector.scalar_tensor_tensor(
                hA[:], axf[:, A_W1:A_W1 + HH], f_pd[:],
                axf[:, A_B1:A_B1 + HH], op0=Alu.mult, op1=Alu.add)
            hB = scr.tile([MS, HH], f32, tag="hB")
            nc.vector.scalar_tensor_tensor(
                hB[:], axf[:, A_W1 + HH:A_W1 + 2 * HH], f_pmax[:], hA[:],
                op0=Alu.mult, op1=Alu.add)
            hC = scr.tile([MS, HH], f32, tag="hC")
            nc.vector.scalar_tensor_tensor(
                hC[:], axf[:, A_W1 + 2 * HH:A_W1 + 3 * HH], f_gap[:], hB[:],
                op0=Alu.mult, op1=Alu.add)
            hD = per.tile([MS, HH], f32, tag="hD")
            nc.vector.scalar_tensor_tensor(
                hD[:], axf[:, A_W1 + 3 * HH:A_W1 + 4 * HH], f_ent[:], hC[:],
                op0=Alu.mult, op1=Alu.add)

            # ---- the fused pool+proj matmul chain ----
            # ps1 partitions [0:33] = group A, [64:97] = group B.
            # The f32 proj_b rank-1 update runs LAST (it waits on the big
            # axf DMA; putting it first stalled the whole chain ~4.4us and
            # let the HAM re-throttle the PE).
            ps1 = psp.tile([CH, BW], f32, tag="ps1", name="ps1")
            started = [False, False]
            for b in range(NBLK):
                h = b // S
                ci, co = b // BPC, (b % BPC) * BW
                grp = (b % 2) if COLT else 0
                po = 0 if grp == 0 else MS
                st_flag = not started[grp]
                started[grp] = True
                # each group closes at its last stream block (96/97); the
                # bias MM slots in mid-stream (axf has landed by then, and
                # at the end it would stall the transposes behind it)
                stop_flag = (b == NBLK - 1) or (COLT and b == NBLK - 2)
                nc.tensor.matmul(
                    ps1[po:po + PP + 1, :],
                    wb[:, h * (PP + 1):(h + 1) * (PP + 1)],
                    chunks[ci][:, co:co + BW],
                    start=st_flag, stop=stop_flag,
                    skip_group_check=True,
                    tile_position=(0, po) if COLT else None)
                if b == 4 * BPC - 1:
                    nc.tensor.matmul(ps1[0:PP + 1, :],
                                     axf[0:1, A_BIAS:A_BIAS + PP + 1],
                                     axf[0:1, A_ONES:A_ONES + BW],
                                     start=False, stop=False,
                                     skip_group_check=True,
                                     tile_position=(0, 0) if COLT else None)

            # ---- psum -> SBUF -> PE-transpose -> ps2[64, NG*2*33] ----
            sb1 = per.tile([CH - 31, BW], f32)
            nc.vector.tensor_copy(sb1[0:PP + 1, :], ps1[0:PP + 1, :])
            if COLT:
                nc.vector.tensor_copy(sb1[MS:MS + PP + 1, :],
                                      ps1[MS:MS + PP + 1, :])
            ps2 = psp.tile([MS, 2 * (PP + 1)], f32, tag="ps2", name="ps2")
            W33 = PP + 1
            # group-B transposes ACCUMULATE onto group A's region, merging
            # the col-tiled halves for free on the PE
            nc.tensor.matmul(ps2[:, 0:W33], sb1[0:W33, 0:MS],
                             idn2[0:W33, :], is_transpose=True,
                             start=True, stop=not COLT,
                             skip_group_check=True, tile_position=(0, 0))
            nc.tensor.matmul(ps2[:, W33:2 * W33], sb1[0:W33, MS:BW],
                             idn2[0:W33, :], is_transpose=True,
                             start=True, stop=not COLT,
                             skip_group_check=True, tile_position=(0, 0))
            if COLT:
                nc.tensor.matmul(ps2[:, 0:W33], sb1[MS:MS + W33, 0:MS],
                                 idn2[MS:MS + W33, :], is_transpose=True,
                                 start=False, stop=True,
                                 skip_group_check=True,
                                 tile_position=(MS, 0))
                nc.tensor.matmul(ps2[:, W33:2 * W33],
                                 sb1[MS:MS + W33, MS:BW],
                                 idn2[MS:MS + W33, :], is_transpose=True,
                                 start=False, stop=True,
                                 skip_group_check=True,
                                 tile_position=(MS, 0))

            # ---- center:  src cols = [v | -mu] per roi, per w-group ----
            src = ps2
            ctr = per.tile([MS, 2 * PP], f32)
            for w in (0, 1):
                po = w * W33
                nc.vector.tensor_scalar(ctr[:, w * PP:(w + 1) * PP],
                                        src[:, po:po + PP],
                                        src[:, po + PP:po + PP + 1],
                                        None, op0=Alu.add)

            if fastpath:
                # ln_b == 0: rstd cancels in the normalized cosine, so
                # cos = <g*ctr_x, g*ctr_v> / (|g*ctr_x| |g*ctr_v|)
                u = per.tile([MS, 2 * PP], f32)
                nc.vector.tensor_tensor(u[:], ctr[:],
                                        axf[:, A_G:A_G + 2 * PP],
                                        op=Alu.mult)
                ss = per.tile([MS, 2], f32)
                jn = scr.tile([MS, 2 * PP], f32, tag="jn")
                for w in (0, 1):
                    us = u[:, w * PP:(w + 1) * PP]
                    nc.vector.scalar_tensor_tensor(
                        jn[:, w * PP:(w + 1) * PP], us, 1.0, us,
                        op0=Alu.bypass, op1=Alu.mult,
                        accum_out=ss[:, w:w + 1])
                dot = per.tile([MS, 1], f32)
                jd = scr.tile([MS, PP], f32, tag="jd")
                nc.vector.scalar_tensor_tensor(jd[:], u[:, 0:PP], 1.0,
                                               u[:, PP:2 * PP],
                                               op0=Alu.bypass, op1=Alu.mult,
                                               accum_out=dot[:])
            else:
                vs = per.tile([MS, 2], f32)
                jv = scr.tile([MS, 2 * PP], f32, tag="jv")
                for w in (0, 1):
                    cs = ctr[:, w * PP:(w + 1) * PP]
                    nc.vector.scalar_tensor_tensor(
                        jv[:, w * PP:(w + 1) * PP], cs, 1.0, cs,
                        op0=Alu.bypass, op1=Alu.mult,
                        accum_out=vs[:, w:w + 1])
                sd = scr.tile([MS, 2], f32, tag="sd")
                nc.scalar.activation(sd[:], vs[:], Act.Sqrt,
                                     scale=1.0 / (PP * SEFF * SEFF),
                                     bias=eln[:])
                rstd = per.tile([MS, 2], f32)
                nc.vector.reciprocal(rstd[:], sd[:])
                gr = scr.tile([MS, 2 * PP], f32, tag="gr")
                for w in (0, 1):
                    nc.vector.tensor_scalar(gr[:, w * PP:(w + 1) * PP],
                                            axf[:, A_G + w * PP:
                                                A_G + (w + 1) * PP],
                                            rstd[:, w:w + 1], None,
                                            op0=Alu.mult)
                yt = scr.tile([MS, 2 * PP], f32, tag="yt")
                nc.vector.tensor_tensor(yt[:], ctr[:], gr[:], op=Alu.mult)
                u = per.tile([MS, 2 * PP], f32)
                nc.vector.tensor_tensor(u[:], yt[:],
                                        axf[:, A_B:A_B + 2 * PP], op=Alu.add)
                ss = per.tile([MS, 2], f32)
                jn = scr.tile([MS, 2 * PP], f32, tag="jn")
                for w in (0, 1):
                    ys = u[:, w * PP:(w + 1) * PP]
                    nc.vector.scalar_tensor_tensor(
                        jn[:, w * PP:(w + 1) * PP], ys, 1.0, ys,
                        op0=Alu.bypass, op1=Alu.mult,
                        accum_out=ss[:, w:w + 1])
                dot = per.tile([MS, 1], f32)
                jd = scr.tile([MS, PP], f32, tag="jd")
                nc.vector.scalar_tensor_tensor(jd[:], u[:, 0:PP], 1.0,
                                               u[:, PP:2 * PP],
                                               op0=Alu.bypass, op1=Alu.mult,
                                               accum_out=dot[:])

            s12 = scr.tile([MS, 1], f32, tag="s12")
            nc.vector.tensor_tensor(s12[:], ss[:, 0:1], ss[:, 1:2],
                                    op=Alu.mult)
            sq = scr.tile([MS, 1], f32, tag="sq")
            nc.scalar.activation(sq[:], s12[:], Act.Sqrt)
            rq = per.tile([MS, 1], f32)
            nc.vector.reciprocal(rq[:], sq[:])
            f_cos = per.tile([MS, 1], f32)
            nc.vector.scalar_tensor_tensor(f_cos[:], dot[:], rq[:], hr9[:],
                                           op0=Alu.mult, op1=Alu.mult)

            # ---- finish MLP ----
            hE = per.tile([MS, HH], f32)
            nc.vector.scalar_tensor_tensor(
                hE[:], axf[:, A_W1 + 4 * HH:A_W1 + 5 * HH], f_cos[:],
                hD[:], op0=Alu.mult, op1=Alu.add)
            hR = per.tile([MS, HH], f32)
            nc.vector.tensor_scalar(hR[:], hE[:], 0.0, None, op0=Alu.max)
            logit = per.tile([MS, 1], f32)
            jl = scr.tile([MS, HH], f32, tag="jl")
            nc.vector.scalar_tensor_tensor(jl[:], hR[:], 1.0,
                                           axf[:, A_W2:A_W2 + HH],
                                           op0=Alu.bypass, op1=Alu.mult,
                                           accum_out=logit[:])
            sg = per.tile([MS, 1], f32)
            nc.scalar.activation(sg[:], logit[:], Act.Sigmoid,
                                 bias=axf[:, A_B2:A_B2 + 1])
            gt = per.tile([MS, 1], f32)
            nc.vector.scalar_tensor_tensor(gt[:], sg[:], 0.999, hr6[:],
                                           op0=Alu.min, op1=Alu.mult)
            res = per.tile([MS, 1], f32)
            nc.vector.tensor_scalar(res[:], gt[:], 0.001, None, op0=Alu.max)

            # ---- transpose result to a [1, 64] row -> single contiguous
            # 256B output DMA (partition-strided 4B writes stall the
            # completion semaphore ~6us) ----
            pout = psp.tile([1, MS], f32, tag="pout", name="pout")
            nc.tensor.matmul(pout[:], res[:],
                             axf[:, A_ID64:A_ID64 + MS],
                             start=True, stop=True, skip_group_check=True)
            rrow = per.tile([1, MS], f32)
            nc.vector.tensor_copy(rrow[:], pout[:])
            nc.sync.dma_start(out=out_d[:], in_=rrow[:])

    nc.finalize()
    return nc


def _get_nc(fastpath):
    key = ("nc", fastpath)
    if key not in _CACHE:
        _CACHE[key] = _build(fastpath)
    return _CACHE[key]


def make_in_maps(x, prev_x, match, proj_w, proj_b, ln_g, ln_b, w1, b1, w2, b2):
    f32 = np.float32
    f16 = np.float16
    x0 = np.asarray(x[0], dtype=f32).reshape(M, C, S)
    p0 = np.asarray(prev_x[0], dtype=f32).reshape(N, C, S)
    mt0 = np.ascontiguousarray(np.asarray(match[0], dtype=f32))
    real0 = mt0[:, :N]
    rm = real0.sum(axis=1)
    top1 = np.where(rm > EPS, np.argmax(real0, axis=1), 0)

    proj_w = np.asarray(proj_w, dtype=f32)   # (32, 256)
    proj_b = np.asarray(proj_b, dtype=f32)

    # stream: [core, 128 chan-half, 98 blocks (h-major) x 128 (64 x | 64 v)]
    def shard_blocks(rows):                  # (512, 256, 49) -> (8,2,49,128,64)
        return (rows.reshape(NCORES, MS, 2, CH, S)
                    .transpose(0, 2, 4, 3, 1))
    xt = shard_blocks(x0 * SX)
    vt = shard_blocks(p0[top1] * SX)
    comb = np.concatenate([xt, vt], axis=4)              # (8,2,49,128,128)
    stream = np.ascontiguousarray(
        comb.transpose(0, 3, 1, 2, 4).reshape(NCORES, CH, NBLK * BW)
    ).astype(F8)

    # weights: per half h, [128, 33]: cols 0:32 = 32*w[:, h*128+c].T,
    # col 32 = -32 * mean_p w  (negated column-mean row for centering)
    wb = np.zeros((CH, 2 * (PP + 1)), dtype=f32)
    for h in (0, 1):
        blk = proj_w[:, h * CH:(h + 1) * CH].T * SW      # (128, 32)
        wb[:, h * (PP + 1):h * (PP + 1) + PP] = blk
        wb[:, h * (PP + 1) + PP] = -blk.mean(axis=1)
    wb = wb.astype(F8)

    idn = np.eye(PP + 1, dtype=f32)

    axf = np.zeros((MS, A_COLS), dtype=f32)
    ln_g = np.asarray(ln_g, dtype=f32)
    ln_b = np.asarray(ln_b, dtype=f32)
    fastpath = bool(np.all(ln_b == 0.0))
    # fastpath cosine is scale-invariant -> raw ln_g; general path folds
    # the stream scale into g (y = ctrS * rstd_true * g/SEFF)
    gfill = ln_g if fastpath else ln_g / SEFF
    axf[:, A_G:A_G + PP] = gfill
    axf[:, A_G + PP:A_G + 2 * PP] = gfill
    axf[:, A_B:A_B + PP] = ln_b
    axf[:, A_B + PP:A_B + 2 * PP] = ln_b
    w1 = np.asarray(w1, dtype=f32)           # (32, 5)
    for f in range(5):
        axf[:, A_W1 + f * HH:A_W1 + (f + 1) * HH] = w1[:, f]
    axf[:, A_B1:A_B1 + HH] = np.asarray(b1, dtype=f32)
    axf[:, A_W2:A_W2 + HH] = np.asarray(w2, dtype=f32)[0]
    axf[:, A_B2] = np.asarray(b2, dtype=f32)[0]
    axf[0, A_BIAS:A_BIAS + PP] = SEFF * proj_b
    axf[0, A_BIAS + PP] = -SEFF * proj_b.mean()
    axf[0, A_ONES:A_ONES + BW] = 1.0
    axf[:, A_ID64:A_ID64 + MS] = np.eye(MS, dtype=f32)

    in_maps = []
    for i in range(NCORES):
        lo, hi = i * MS, (i + 1) * MS
        in_maps.append({
            "st": stream[i],
            "mt": np.ascontiguousarray(mt0[lo:hi]).astype(f16),
            "wb": wb, "idn": idn, "axf": axf,
        })
    return in_maps, fastpath


def run(in_maps, fastpath=True, trace=False):
    from concourse.bass_utils import run_bass_kernel_spmd
    res = run_bass_kernel_spmd(_get_nc(fastpath), in_maps,
                               list(range(NCORES)), trace=trace)
    out = np.concatenate(
        [res.results[i]["out"].reshape(MS, 1) for i in range(NCORES)], axis=0)
    if trace:
        print("dbg sentinel (expect 30s):", res.results[0]["dbg"])
    return out.astype(np.float32), res


def kernel(x, prev_x, match, proj_w, proj_b, ln_g, ln_b, w1, b1, w2, b2):
    in_maps, fastpath = make_in_maps(x, prev_x, match, proj_w, proj_b,
                                     ln_g, ln_b, w1, b1, w2, b2)
    out, _ = run(in_maps, fastpath=fastpath, trace=False)
    return out


# revision 37
# speedup vs baseline: 1.1974x; 1.0631x over previous
"""ConfidenceGate Trainium2 kernel (8 NeuronCores, SPMD) — v4.

Problem recap (shapes hardcoded from the spec):
  x:      (4, 512, 256, 7, 7) f32
  prev_x: (4, 512, 256, 7, 7) f32
  match:  (4, 512, 513) f32
  + tiny proj/LN/MLP params.
Reference returns c[0] -> (512, 1): only batch 0 contributes to the output.

Strategy (v4 — PE-fused streaming, col-tiled, fast epilogue):
  * Batch 0 only; data-parallel over M=512 rois: 8 cores x 64.
  * top1 = argmax(match[0,:,:512]) on host; prev rows pre-gathered per shard.
  * Pooling AND projection fused into accumulating matmuls on the Tensor
    engine: 98 blocks of [128 chan-half, 128 rois(64 x | 64 v)] in fp8
    (x*8, w*32 scaling; scales folded into the epilogue).  Weights
    ([128, 33] incl a negated-column-mean column for free centering) are
    stationary; even/odd blocks go to PE col-groups (0,0)/(0,64) so two
    streams run concurrently on separate sub-arrays.  proj_b rides in as
    a K=1 f32 rank-1 matmul into group A.
  * PE pre-warmed with junk matmuls (HAM un-throttle) during the DMA fill;
    gpsimd memsets run BEFORE its SWDGE dma_starts (the SWDGE drain
    otherwise serializes them behind DMA completion).
  * ln_b == 0 fast path: per-roi rstd cancels exactly inside the
    normalized cosine, so the LN variance/sqrt/scale chain is skipped
    (u = g*(v-mu) feeds the cosine directly).  General path kept for
    nonzero ln_b.
  * result is PE-transposed to a [1, 64] row so the output DMA is one
    contiguous 256B descriptor (64 partition-strided 4B writes cost ~6us
    in completion-semaphore latency).
  * match stats ([64,512] fp16) on DVE, overlapped with the stream; 4 of
    5 MLP features pre-accumulated.  ACT tables: Ln early, Sqrt primed
    mid-stream, Sigmoid load hidden under the MLP DVE ops.
"""

import sys

if "/opt/trn_rl_repo" not in sys.path:
    sys.path.insert(0, "/opt/trn_rl_repo")

import numpy as np
import ml_dtypes

B, M, N, C, G = 4, 512, 512, 256, 7
S = G * G                      # 49 spatial positions
PP, HH = 32, 32                # proj dim, MLP hidden
NCORES = 8
MS = M // NCORES               # 64 rois per core
CH = C // 2                    # 128 channels per half = contract dim
NBLK = 2 * S                   # 98 matmul blocks (h-major: b = h*49 + s)
BW = 2 * MS                    # 128 cols per block (64 x | 64 v)
# chunk sizes in blocks: small first chunk so the MM chain starts early;
# all on the sync HWDGE queue (FIFO -> chunks complete in consumption order)
CHUNK_BLKS = [10, 22, 22, 22, 22]
SX, SW = 8.0, 32.0             # fp8 scales for data / weights
SEFF = S * SX * SW             # 12544 = total scale on psum values
EPS = 1e-9
LN_EPS = 1e-5
NEG = -60000.0                 # fp16-safe "-inf" for second-max masking
NPRE = 2                       # PE prewarm matmuls (N=512 each, ~0.75us cold)
COLT = True                    # 2x PE column tiling (groups at col 0 / 64)

F8 = ml_dtypes.float8_e4m3

# axf (f32) column layout, 64 partitions (one row per roi)
A_G = 0                        # ln_g / SEFF replicated twice (64, 64) [x|v]
A_B = 64                       # ln_b replicated twice (64, 64) for [x|v]
A_W1 = 128                     # w1 block f at [128+32f : 160+32f), f=0..4
A_B1 = 288                     # b1 replicated (64, 32)
A_W2 = 320                     # w2[0] replicated (64, 32)
A_B2 = 352                     # b2 (64, 1)
A_BIAS = 353                   # SEFF*(proj_b | -mean) (row 0 only, 33)
A_ONES = 386                   # 1.0 x 128 (row 0 only)
A_ID64 = 514                   # 64x64 identity (rows 0..63)
A_COLS = 578

_CACHE = {}


def _build(fastpath):
    import concourse.bacc as bacc
    import concourse.tile as tile
    import concourse.mybir as mybir

    dt = mybir.dt
    Alu = mybir.AluOpType
    Act = mybir.ActivationFunctionType
    Ax = mybir.AxisListType
    f32 = dt.float32
    f16 = dt.float16
    f8 = dt.float8e4

    nc = bacc.Bacc("TRN2", target_bir_lowering=False, debug=False)

    st_d = nc.dram_tensor("st", [CH, NBLK * BW], f8, kind="ExternalInput")
    mt_d = nc.dram_tensor("mt", [MS, N + 1], f16, kind="ExternalInput")
    wb_d = nc.dram_tensor("wb", [CH, 2 * (PP + 1)], f8, kind="ExternalInput")
    id_d = nc.dram_tensor("idn", [PP + 1, PP + 1], f32, kind="ExternalInput")
    axf_d = nc.dram_tensor("axf", [MS, A_COLS], f32, kind="ExternalInput")
    out_d = nc.dram_tensor("out", [1, MS], f32, kind="ExternalOutput")
    dbg_d = nc.dram_tensor("dbg", [1, 8], f32, kind="ExternalOutput")

    NG = 2 if COLT else 1      # psum col groups

    with tile.TileContext(nc) as tc, nc.allow_low_precision(
        reason="fp8 pool+proj stream + fp16 match stats; logit margin ~0.79 "
        "vs ~1e-3 observed logit err (validated on host)"
    ):
        with (
            tc.tile_pool(name="persist", bufs=1) as per,
            tc.tile_pool(name="chunks", bufs=1) as big,
            tc.tile_pool(name="scratch", bufs=1) as scr,
            tc.tile_pool(name="psum", bufs=1, space="PSUM") as psp,
        ):
            # ---- small loads on the scalar (ACT) HWDGE ring; wb first
            # (gates matmuls), then axf (bias/identities), then mt ----
            mt = per.tile([MS, N + 1], f16)
            nc.scalar.dma_start(out=mt[:], in_=mt_d[:])
            wb = per.tile([CH, 2 * (PP + 1)], f8)
            nc.scalar.dma_start(out=wb[:], in_=wb_d[:])
            axf = per.tile([MS, A_COLS], f32)
            nc.scalar.dma_start(out=axf[:], in_=axf_d[:])
            # 33x33 identity replicated at partitions 0-32 and 64-96 for
            # the group-B transposes (rhs must live on the B row range)
            idn2 = per.tile([CH - 31, PP + 1], f32)
            nc.scalar.dma_start(out=idn2[0:PP + 1, :], in_=id_d[:])
            if COLT:
                nc.scalar.dma_start(out=idn2[MS:MS + PP + 1, :], in_=id_d[:])

            # ---- gpsimd: ALL memsets FIRST (before its SWDGE dma_starts,
            # which drain-block anything queued after them) ----
            e9 = per.tile([MS, 1], f32)
            nc.gpsimd.memset(e9[:], EPS)
            dmy = scr.tile([1, 1], f32, tag="dmy")
            nc.gpsimd.memset(dmy[:], 1.0)
            pr_in = scr.tile([1, 8], f32, tag="prin")
            nc.gpsimd.memset(pr_in[:], 30.0)
            if not fastpath:
                eln = per.tile([MS, 1], f32)
                nc.gpsimd.memset(eln[:], LN_EPS)

            # ---- big stream chunks, all on the sync HWDGE queue: FIFO
            # drain means chunk i completes before chunk i+1, matching MM
            # consumption order (splitting across queues starves chunk0) ----
            chunks = []       # (tile, first_block)
            b0 = 0
            for i, nb in enumerate(CHUNK_BLKS):
                t = big.tile([CH, nb * BW], f8, tag=f"ch{i}", name=f"ch{i}")
                c0 = b0 * BW
                nc.sync.dma_start(out=t[:], in_=st_d[:, c0:c0 + nb * BW])
                chunks.append((t, b0))
                b0 += nb

            # ---- PE prewarm (HAM un-throttle during DMA fill). junk is
            # memset on the DVE (idle until mt arrives) so prewarm starts
            # right after the preamble ----
            junk = per.tile([CH, 512], f16)
            nc.vector.memset(junk[:], 0.25)
            jps = psp.tile([PP + 1, 512], f32, tag="jps", name="jps")
            for _ in range(NPRE):
                nc.tensor.matmul(jps[:], junk[:, 0:PP + 1], junk[:],
                                 start=True, stop=True, skip_group_check=True)

            # ---- dbg sentinel ----
            prb = per.tile([1, 8], f32)
            nc.vector.tensor_scalar(prb[:], pr_in[:], 1.0, None, op0=Alu.mult)
            nc.scalar.dma_start(out=dbg_d[:], in_=prb[:])

            # ---- match stats on DVE (overlap the stream) ----
            real = mt[:, 0:N]
            pd = mt[:, N:N + 1]
            rmass = per.tile([MS, 1], f32)
            jr = scr.tile([MS, N], f16, tag="jr")
            nc.vector.tensor_scalar(jr[:], real, 1.0, 0.0, op0=Alu.mult,
                                    op1=Alu.add, accum_out=rmass[:])
            f_pmax = per.tile([MS, 1], f32)
            nc.vector.reduce_max(f_pmax[:], real, axis=Ax.X)
            eqm = scr.tile([MS, N], f16, tag="eqm")
            nc.vector.tensor_scalar(eqm[:], real, f_pmax[:], None,
                                    op0=Alu.is_equal)
            msk = scr.tile([MS, N], f16, tag="msk")
            nc.vector.scalar_tensor_tensor(msk[:], eqm[:], NEG, real,
                                           op0=Alu.mult, op1=Alu.add)
            m2 = per.tile([MS, 1], f32)
            nc.vector.reduce_max(m2[:], msk[:], axis=Ax.X)
            f_gap = per.tile([MS, 1], f32)
            nc.vector.tensor_tensor(f_gap[:], f_pmax[:], m2[:],
                                    op=Alu.subtract)
            # entropy feature is -ent = sum(real * ln(real+eps)) directly
            lnr = scr.tile([MS, N], f16, tag="lnr")
            nc.scalar.activation(lnr[:], real, Act.Ln, bias=e9[:])
            f_ent = per.tile([MS, 1], f32)
            je = scr.tile([MS, N], f16, tag="je")
            nc.vector.scalar_tensor_tensor(je[:], real, 1.0, lnr[:],
                                           op0=Alu.bypass, op1=Alu.mult,
                                           accum_out=f_ent[:])
            # prime the Sqrt table while the stream is flowing (priming
            # Sigmoid too backfires: 4 live sets -> walrus emits 6 loads)
            pre = scr.tile([1, 1], f32, tag="pre")
            nc.scalar.activation(pre[:], dmy[:], Act.Sqrt)
            f_pd = per.tile([MS, 1], f32)
            nc.vector.tensor_scalar(f_pd[:], pd, -1.0, 1.0, op0=Alu.mult,
                                    op1=Alu.add)
            hr9 = per.tile([MS, 1], f32)
            nc.vector.tensor_scalar(hr9[:], rmass[:], EPS, None, op0=Alu.is_gt)
            hr6 = per.tile([MS, 1], f32)
            nc.vector.tensor_scalar(hr6[:], rmass[:], 1e-6, None,
                                    op0=Alu.is_gt)

            # ---- MLP hidden pre-accumulation over the 4 early features ----
            hA = scr.tile([MS, HH], f32, tag="hA")
            nc.vector.scalar_tensor_tensor(
                hA[:], axf[:, A_W1:A_W1 + HH], f_pd[:],
                axf[:, A_B1:A_B1 + HH], op0=Alu.mult, op1=Alu.add)
            hB = scr.tile([MS, HH], f32, tag="hB")
            nc.vector.scalar_tensor_tensor(
                hB[:], axf[:, A_W1 + HH:A_W1 + 2 * HH], f_pmax[:], hA[:],
                op0=Alu.mult, op1=Alu.add)
            hC = scr.tile([MS, HH], f32, tag="hC")
            nc.vector.scalar_tensor_tensor(
                hC[:], axf[:, A_W1 + 2 * HH:A_W1 + 3 * HH], f_gap[:], hB[:],
                op0=Alu.mult, op1=Alu.add)
            hD = per.tile([MS, HH], f32, tag="hD")
            nc.vector.scalar_tensor_tensor(
                hD[:], axf[:, A_W1 + 3 * HH:A_W1 + 4 * HH], f_ent[:], hC[:],
                op0=Alu.mult, op1=Alu.add)

            # ---- the fused pool+proj matmul chain ----
            # ps1 partitions [0:33] = group A, [64:97] = group B.
            # The f32 proj_b rank-1 update runs LAST (it waits on the big
            # axf DMA; putting it first stalled the whole chain ~4.4us and
            # let the HAM re-throttle the PE).
            ps1 = psp.tile([CH, BW], f32, tag="ps1", name="ps1")
            blk2chunk = {}
            for ci, (t, fb) in enumerate(chunks):
                for j in range(CHUNK_BLKS[ci]):
                    blk2chunk[fb + j] = (ci, j)
            started = [False, False]
            for b in range(NBLK):
                h = b // S
                ci, j = blk2chunk[b]
                co = j * BW
                grp = (b % 2) if COLT else 0
                po = 0 if grp == 0 else MS
                st_flag = not started[grp]
                started[grp] = True
                # each group closes at its last stream block (96/97); the
                # bias MM slots in mid-stream (axf has landed by then, and
                # at the end it would stall the transposes behind it)
                stop_flag = (b == NBLK - 1) or (COLT and b == NBLK - 2)
                nc.tensor.matmul(
                    ps1[po:po + PP + 1, :],
                    wb[:, h * (PP + 1):(h + 1) * (PP + 1)],
                    chunks[ci][0][:, co:co + BW],
                    start=st_flag, stop=stop_flag,
                    skip_group_check=True,
                    tile_position=(0, po) if COLT else None)
                if b == 55:
                    nc.tensor.matmul(ps1[0:PP + 1, :],
                                     axf[0:1, A_BIAS:A_BIAS + PP + 1],
                                     axf[0:1, A_ONES:A_ONES + BW],
                                     start=False, stop=False,
                                     skip_group_check=True,
                                     tile_position=(0, 0) if COLT else None)

            # ---- psum -> SBUF -> PE-transpose -> ps2[64, NG*2*33] ----
            sb1 = per.tile([CH - 31, BW], f32)
            nc.vector.tensor_copy(sb1[0:PP + 1, :], ps1[0:PP + 1, :])
            if COLT:
                nc.vector.tensor_copy(sb1[MS:MS + PP + 1, :],
                                      ps1[MS:MS + PP + 1, :])
            ps2 = psp.tile([MS, 2 * (PP + 1)], f32, tag="ps2", name="ps2")
            W33 = PP + 1
            # group-B transposes ACCUMULATE onto group A's region, merging
            # the col-tiled halves for free on the PE
            nc.tensor.matmul(ps2[:, 0:W33], sb1[0:W33, 0:MS],
                             idn2[0:W33, :], is_transpose=True,
                             start=True, stop=not COLT,
                             skip_group_check=True, tile_position=(0, 0))
            nc.tensor.matmul(ps2[:, W33:2 * W33], sb1[0:W33, MS:BW],
                             idn2[0:W33, :], is_transpose=True,
                             start=True, stop=not COLT,
                             skip_group_check=True, tile_position=(0, 0))
            if COLT:
                nc.tensor.matmul(ps2[:, 0:W33], sb1[MS:MS + W33, 0:MS],
                                 idn2[MS:MS + W33, :], is_transpose=True,
                                 start=False, stop=True,
                                 skip_group_check=True,
                                 tile_position=(MS, 0))
                nc.tensor.matmul(ps2[:, W33:2 * W33],
                                 sb1[MS:MS + W33, MS:BW],
                                 idn2[MS:MS + W33, :], is_transpose=True,
                                 start=False, stop=True,
                                 skip_group_check=True,
                                 tile_position=(MS, 0))

            # ---- center:  src cols = [v | -mu] per roi, per w-group ----
            src = ps2
            ctr = per.tile([MS, 2 * PP], f32)
            for w in (0, 1):
                po = w * W33
                nc.vector.tensor_scalar(ctr[:, w * PP:(w + 1) * PP],
                                        src[:, po:po + PP],
                                        src[:, po + PP:po + PP + 1],
                                        None, op0=Alu.add)

            if fastpath:
                # ln_b == 0: rstd cancels in the normalized cosine, so
                # cos = <g*ctr_x, g*ctr_v> / (|g*ctr_x| |g*ctr_v|)
                u = per.tile([MS, 2 * PP], f32)
                nc.vector.tensor_tensor(u[:], ctr[:],
                                        axf[:, A_G:A_G + 2 * PP],
                                        op=Alu.mult)
                ss = per.tile([MS, 2], f32)
                jn = scr.tile([MS, 2 * PP], f32, tag="jn")
                for w in (0, 1):
                    us = u[:, w * PP:(w + 1) * PP]
                    nc.vector.scalar_tensor_tensor(
                        jn[:, w * PP:(w + 1) * PP], us, 1.0, us,
                        op0=Alu.bypass, op1=Alu.mult,
                        accum_out=ss[:, w:w + 1])
                dot = per.tile([MS, 1], f32)
                jd = scr.tile([MS, PP], f32, tag="jd")
                nc.vector.scalar_tensor_tensor(jd[:], u[:, 0:PP], 1.0,
                                               u[:, PP:2 * PP],
                                               op0=Alu.bypass, op1=Alu.mult,
                                               accum_out=dot[:])
            else:
                vs = per.tile([MS, 2], f32)
                jv = scr.tile([MS, 2 * PP], f32, tag="jv")
                for w in (0, 1):
                    cs = ctr[:, w * PP:(w + 1) * PP]
                    nc.vector.scalar_tensor_tensor(
                        jv[:, w * PP:(w + 1) * PP], cs, 1.0, cs,
                        op0=Alu.bypass, op1=Alu.mult,
                        accum_out=vs[:, w:w + 1])
                sd = scr.tile([MS, 2], f32, tag="sd")
                nc.scalar.activation(sd[:], vs[:], Act.Sqrt,
                                     scale=1.0 / (PP * SEFF * SEFF),
                                     bias=eln[:])
                rstd = per.tile([MS, 2], f32)
                nc.vector.reciprocal(rstd[:], sd[:])
                gr = scr.tile([MS, 2 * PP], f32, tag="gr")
                for w in (0, 1):
                    nc.vector.tensor_scalar(gr[:, w * PP:(w + 1) * PP],
                                            axf[:, A_G + w * PP:
                                                A_G + (w + 1) * PP],
                                            rstd[:, w:w + 1], None,
                                            op0=Alu.mult)
                yt = scr.tile([MS, 2 * PP], f32, tag="yt")
                nc.vector.tensor_tensor(yt[:], ctr[:], gr[:], op=Alu.mult)
                u = per.tile([MS, 2 * PP], f32)
                nc.vector.tensor_tensor(u[:], yt[:],
                                        axf[:, A_B:A_B + 2 * PP], op=Alu.add)
                ss = per.tile([MS, 2], f32)
                jn = scr.tile([MS, 2 * PP], f32, tag="jn")
                for w in (0, 1):
                    ys = u[:, w * PP:(w + 1) * PP]
                    nc.vector.scalar_tensor_tensor(
                        jn[:, w * PP:(w + 1) * PP], ys, 1.0, ys,
                        op0=Alu.bypass, op1=Alu.mult,
                        accum_out=ss[:, w:w + 1])
                dot = per.tile([MS, 1], f32)
                jd = scr.tile([MS, PP], f32, tag="jd")
                nc.vector.scalar_tensor_tensor(jd[:], u[:, 0:PP], 1.0,
                                               u[:, PP:2 * PP],
                                               op0=Alu.bypass, op1=Alu.mult,
                                               accum_out=dot[:])

            s12 = scr.tile([MS, 1], f32, tag="s12")
            nc.vector.tensor_tensor(s12[:], ss[:, 0:1], ss[:, 1:2],
                                    op=Alu.mult)
            sq = scr.tile([MS, 1], f32, tag="sq")
            nc.scalar.activation(sq[:], s12[:], Act.Sqrt)
            rq = per.tile([MS, 1], f32)
            nc.vector.reciprocal(rq[:], sq[:])
            f_cos = per.tile([MS, 1], f32)
            nc.vector.scalar_tensor_tensor(f_cos[:], dot[:], rq[:], hr9[:],
                                           op0=Alu.mult, op1=Alu.mult)

            # ---- finish MLP ----
            hE = per.tile([MS, HH], f32)
            nc.vector.scalar_tensor_tensor(
                hE[:], axf[:, A_W1 + 4 * HH:A_W1 + 5 * HH], f_cos[:],
                hD[:], op0=Alu.mult, op1=Alu.add)
            hR = per.tile([MS, HH], f32)
            nc.vector.tensor_scalar(hR[:], hE[:], 0.0, None, op0=Alu.max)
            logit = per.tile([MS, 1], f32)
            jl = scr.tile([MS, HH], f32, tag="jl")
            nc.vector.scalar_tensor_tensor(jl[:], hR[:], 1.0,
                                           axf[:, A_W2:A_W2 + HH],
                                           op0=Alu.bypass, op1=Alu.mult,
                                           accum_out=logit[:])
            sg = per.tile([MS, 1], f32)
            nc.scalar.activation(sg[:], logit[:], Act.Sigmoid,
                                 bias=axf[:, A_B2:A_B2 + 1])
            gt = per.tile([MS, 1], f32)
            nc.vector.scalar_tensor_tensor(gt[:], sg[:], 0.999, hr6[:],
                                           op0=Alu.min, op1=Alu.mult)
            res = per.tile([MS, 1], f32)
            nc.vector.tensor_scalar(res[:], gt[:], 0.001, None, op0=Alu.max)

            # ---- transpose result to a [1, 64] row -> single contiguous
            # 256B output DMA (partition-strided 4B writes stall the
            # completion semaphore ~6us) ----
            pout = psp.tile([1, MS], f32, tag="pout", name="pout")
            nc.tensor.matmul(pout[:], res[:],
                             axf[:, A_ID64:A_ID64 + MS],
                             start=True, stop=True, skip_group_check=True)
            rrow = per.tile([1, MS], f32)
            nc.vector.tensor_copy(rrow[:], pout[:])
            nc.sync.dma_start(out=out_d[:], in_=rrow[:])

    nc.finalize()
    return nc


def _get_nc(fastpath):
    key = ("nc", fastpath)
    if key not in _CACHE:
        _CACHE[key] = _build(fastpath)
    return _CACHE[key]


def make_in_maps(x, prev_x, match, proj_w, proj_b, ln_g, ln_b, w1, b1, w2, b2):
    f32 = np.float32
    f16 = np.float16
    x0 = np.asarray(x[0], dtype=f32).reshape(M, C, S)
    p0 = np.asarray(prev_x[0], dtype=f32).reshape(N, C, S)
    mt0 = np.ascontiguousarray(np.asarray(match[0], dtype=f32))
    real0 = mt0[:, :N]
    rm = real0.sum(axis=1)
    top1 = np.where(rm > EPS, np.argmax(real0, axis=1), 0)

    proj_w = np.asarray(proj_w, dtype=f32)   # (32, 256)
    proj_b = np.asarray(proj_b, dtype=f32)

    # stream: [core, 128 chan-half, 98 blocks (h-major) x 128 (64 x | 64 v)]
    def shard_blocks(rows):                  # (512, 256, 49) -> (8,2,49,128,64)
        return (rows.reshape(NCORES, MS, 2, CH, S)
                    .transpose(0, 2, 4, 3, 1))
    xt = shard_blocks(x0 * SX)
    vt = shard_blocks(p0[top1] * SX)
    comb = np.concatenate([xt, vt], axis=4)              # (8,2,49,128,128)
    stream = np.ascontiguousarray(
        comb.transpose(0, 3, 1, 2, 4).reshape(NCORES, CH, NBLK * BW)
    ).astype(F8)

    # weights: per half h, [128, 33]: cols 0:32 = 32*w[:, h*128+c].T,
    # col 32 = -32 * mean_p w  (negated column-mean row for centering)
    wb = np.zeros((CH, 2 * (PP + 1)), dtype=f32)
    for h in (0, 1):
        blk = proj_w[:, h * CH:(h + 1) * CH].T * SW      # (128, 32)
        wb[:, h * (PP + 1):h * (PP + 1) + PP] = blk
        wb[:, h * (PP + 1) + PP] = -blk.mean(axis=1)
    wb = wb.astype(F8)

    idn = np.eye(PP + 1, dtype=f32)

    axf = np.zeros((MS, A_COLS), dtype=f32)
    ln_g = np.asarray(ln_g, dtype=f32)
    ln_b = np.asarray(ln_b, dtype=f32)
    fastpath = bool(np.all(ln_b == 0.0))
    # fastpath cosine is scale-invariant -> raw ln_g; general path folds
    # the stream scale into g (y = ctrS * rstd_true * g/SEFF)
    gfill = ln_g if fastpath else ln_g / SEFF
    axf[:, A_G:A_G + PP] = gfill
    axf[:, A_G + PP:A_G + 2 * PP] = gfill
    axf[:, A_B:A_B + PP] = ln_b
    axf[:, A_B + PP:A_B + 2 * PP] = ln_b
    w1 = np.asarray(w1, dtype=f32)           # (32, 5)
    for f in range(5):
        axf[:, A_W1 + f * HH:A_W1 + (f + 1) * HH] = w1[:, f]
    axf[:, A_B1:A_B1 + HH] = np.asarray(b1, dtype=f32)
    axf[:, A_W2:A_W2 + HH] = np.asarray(w2, dtype=f32)[0]
    axf[:, A_B2] = np.asarray(b2, dtype=f32)[0]
    axf[0, A_BIAS:A_BIAS + PP] = SEFF * proj_b
    axf[0, A_BIAS + PP] = -SEFF * proj_b.mean()
    axf[0, A_ONES:A_ONES + BW] = 1.0
    axf[:, A_ID64:A_ID64 + MS] = np.eye(MS, dtype=f32)

    in_maps = []
    for i in range(NCORES):
        lo, hi = i * MS, (i + 1) * MS
        in_maps.append({
            "st": stream[i],
            "mt": np.ascontiguousarray(mt0[lo:hi]).astype(f16),
            "wb": wb, "idn": idn, "axf": axf,
        })
    return in_maps, fastpath


def run(in_maps, fastpath=True, trace=False):
    from concourse.bass_utils import run_bass_kernel_spmd
    res = run_bass_kernel_spmd(_get_nc(fastpath), in_maps,
                               list(range(NCORES)), trace=trace)
    out = np.concatenate(
        [res.results[i]["out"].reshape(MS, 1) for i in range(NCORES)], axis=0)
    if trace:
        print("dbg sentinel (expect 30s):", res.results[0]["dbg"])
    return out.astype(np.float32), res


def kernel(x, prev_x, match, proj_w, proj_b, ln_g, ln_b, w1, b1, w2, b2):
    in_maps, fastpath = make_in_maps(x, prev_x, match, proj_w, proj_b,
                                     ln_g, ln_b, w1, b1, w2, b2)
    out, _ = run(in_maps, fastpath=fastpath, trace=False)
    return out
